# revision 28
# baseline (speedup 1.0000x reference)
"""MeshUnpool kernel for 8 trn2 NeuronCores.

Reference semantics (per mesh b):
    v = zeros([N_FULL, F]); v[mask_idx[b]] = images[b]
    for (f, t) in reversed(order[b].T): v[t] = v[f]   # sequential row copies

The sequential copy chain only moves row *provenance* around: after the chain,
every output row i equals v0[src[i]] for a provenance map src that depends only
on `order` (O(K) scalar ops, done on host). Composing with the mask scatter,
out[b, i] = images[b, g[i]] where g[i] = inv_mask[src[i]] or "zero" if src[i]
was never filled. The device work is therefore a pure row-gather (memory-bound).

Sharding: batch (4 meshes) x output-half -> 8 cores. Each core gathers 32768
output rows of 256 f32 from its mesh's 32768-row image table with bulk SWDGE
``dma_gather`` ops (1024 rows per instruction -- the SWDGE descriptor ring
holds 1024 descriptors, which bounds indices per gather), then streams tiles
back to DRAM with contiguous HWDGE writes. The gather's stream->partition
mapping (i -> partition i%128, segment i//128) is inverted host-side in the
index layout so the writeback is a plain contiguous DMA.

int16 indices cover exactly 0..32767, so there are no spare indices for zero
rows: the top ZN=1024 table rows are sacrificed as zero rows (zero-reads must
be spread across many HBM banks -- a single hot row serializes the SDMA
engines, measured 3x slower), and the ~hundreds of output rows that genuinely
reference a sacrificed image row are patched on the host.
"""

import numpy as np

B = 4
N_FULL = 65536
N_SMALL = 32768
K = 32768
F = 256
P = 128
N_CORES = 8
HALF = N_FULL // 2          # output rows per core
NI = 1024                   # rows per dma_gather (SWDGE ring capacity)
SEG = NI // P               # segments per partition per op (8)
OPS = HALF // NI            # 32 ops per core
NBUF = 8                    # data tile double-buffering depth
ZN = 1024                   # sacrificed zero rows (spreads zero-reads across
ZBASE = N_SMALL - ZN        # HBM banks; one hot row serializes the SDMAs)

_prog_cache = {}


def _build_program(reps: int = 1, n_queues: int = 1):
    import concourse.bacc as bacc
    import concourse.tile as tile
    import concourse.mybir as mybir

    nc = bacc.Bacc(
        "TRN2", target_bir_lowering=False, num_swdge_queues=n_queues
    )
    img = nc.dram_tensor("img", [N_SMALL, F], mybir.dt.float32, kind="ExternalInput")
    gidx = nc.dram_tensor(
        "gidx", [P, OPS * NI // 16], mybir.dt.int16, kind="ExternalInput"
    )
    out = nc.dram_tensor("out", [HALF, F], mybir.dt.float32, kind="ExternalOutput")

    with tile.TileContext(nc) as tc:
        with (
            tc.tile_pool(name="data", bufs=NBUF) as pool,
            tc.tile_pool(name="idx", bufs=1) as ipool,
        ):
            idx_all = ipool.tile([P, OPS * NI // 16], mybir.dt.int16)
            nc.gpsimd.dma_start(out=idx_all[:], in_=gidx[:])
            for t in [u % OPS for u in range(reps * OPS)]:
                data = pool.tile([P, SEG * F], mybir.dt.float32)
                nc.gpsimd.dma_gather(
                    out_ap=data[:].rearrange("p (s f) -> p s f", f=F),
                    in_ap=img[:],
                    idxs_ap=idx_all[:, t * (NI // 16) : (t + 1) * (NI // 16)],
                    num_idxs=NI,
                    num_idxs_reg=NI,
                    elem_size=F,
                    queue_num=(t + 1) % n_queues,
                )
                # idx layout already inverted the gather's stream mapping, so
                # SBUF (p, s) holds output row t*NI + p*SEG + s: plain copy.
                nc.sync.dma_start(
                    out=out[t * NI : (t + 1) * NI, :].rearrange(
                        "(p s) f -> p (s f)", p=P
                    ),
                    in_=data[:],
                )
    nc.compile()  # EVSEM legalization: walrus allows 1 sync wait per inst
    return nc


def _get_program():
    # Two SWDGE queues: gathers alternate descriptor rings (ring capacity is
    # 1024 descriptors, exactly one gather), overlapping desc-gen with drain.
    # Measured ~218us/core vs ~288us single-queue.
    if "nc" not in _prog_cache:
        _prog_cache["nc"] = _build_program(n_queues=2)
    return _prog_cache["nc"]


def _provenance(order_b: np.ndarray) -> np.ndarray:
    """src[i] = index into the initial scattered array that ends up at slot i."""
    fs = order_b[0, ::-1].tolist()
    ts = order_b[1, ::-1].tolist()
    src = list(range(N_FULL))
    for f, t in zip(fs, ts):
        src[t] = src[f]
    return np.asarray(src, dtype=np.int64)


def _idx_layout(g: np.ndarray) -> np.ndarray:
    """[HALF] row indices -> [P, OPS*NI/16] wrapped int16 dma_gather layout.

    Stream position i of op t must carry output row t*NI + (i%P)*SEG + i//P
    (so the writeback is contiguous); dma_gather reads stream position i from
    partition i%16, column i//16 (replicated across the 8 Q7 core groups).
    """
    stream = g.reshape(OPS, P, SEG).transpose(0, 2, 1).reshape(OPS, NI)
    w = stream.reshape(OPS, NI // 16, 16).transpose(2, 0, 1).reshape(16, -1)
    return np.ascontiguousarray(np.tile(w, (P // 16, 1)).astype(np.int16))


def _prep_mesh(images_b, mask_idx_b, order_b):
    """Returns (table, g_dev [N_FULL] int, patch_rows, patch_vals)."""
    src = _provenance(order_b)
    inv = np.full(N_FULL, -1, dtype=np.int32)
    inv[mask_idx_b] = np.arange(N_SMALL, dtype=np.int32)
    g0 = inv[src]  # [N_FULL]; -1 where the output row is zero
    # outputs that truly reference a sacrificed image row: patched on host
    patch_rows = np.where(g0 >= ZBASE)[0]
    patch_vals = images_b[g0[patch_rows]]
    # zero outputs gather from zero row ZBASE + (i % ZN): within each 1024-row
    # gather op every zero row is used at most once -> no hot HBM bank
    idx = np.arange(N_FULL, dtype=np.int32)
    g_dev = np.where(g0 < 0, ZBASE + (idx % ZN), g0)
    table = np.ascontiguousarray(images_b).copy()
    table[ZBASE:] = 0.0
    return table, g_dev, patch_rows, patch_vals


def _build_in_maps(images, mask_idx, order):
    images = np.ascontiguousarray(np.asarray(images, dtype=np.float32))
    mask_idx = np.asarray(mask_idx, dtype=np.int64)
    order = np.asarray(order, dtype=np.int64)
    assert images.shape == (B, N_SMALL, F)

    in_maps, patches = [], []
    for b in range(B):
        table, g_dev, patch_rows, patch_vals = _prep_mesh(
            images[b], mask_idx[b], order[b]
        )
        patches.append((patch_rows, patch_vals))
        for h in range(2):
            in_maps.append(
                {
                    "img": table,
                    "gidx": _idx_layout(g_dev[h * HALF : (h + 1) * HALF]),
                }
            )
    return in_maps, patches


def kernel(images, mask_idx, order, n_full, **run_kwargs):
    from concourse.bass_utils import run_bass_kernel_spmd

    assert int(n_full) == N_FULL
    in_maps, patches = _build_in_maps(images, mask_idx, order)
    res = run_bass_kernel_spmd(
        _get_program(), in_maps, core_ids=list(range(N_CORES)), **run_kwargs
    )
    outs = [res.results[c]["out"] for c in range(N_CORES)]
    full = np.stack(
        [np.concatenate([outs[2 * b], outs[2 * b + 1]], axis=0) for b in range(B)]
    )
    for b, (patch_rows, patch_vals) in enumerate(patches):
        if len(patch_rows):
            full[b, patch_rows] = patch_vals
    if run_kwargs:
        kernel.last_results = res
    return full


# revision 31
# speedup vs baseline: 1.5611x; 1.5611x over previous
"""MeshUnpool kernel for 8 trn2 NeuronCores.

Reference semantics (per mesh b):
    v = zeros([N_FULL, F]); v[mask_idx[b]] = images[b]
    for (f, t) in reversed(order[b].T): v[t] = v[f]   # sequential row copies

The sequential copy chain only moves row *provenance* around: after the chain,
every output row i equals v0[src[i]] for a provenance map src that depends only
on `order` (O(K) scalar ops, done on host). Composing with the mask scatter,
out[b, i] = images[b, g[i]] where g[i] = inv_mask[src[i]] or "zero" if src[i]
was never filled. The device work is therefore a pure row-gather (memory-bound).

Sharding: batch (4 meshes) x output-half -> 8 cores. Each core gathers 32768
output rows of 256 f32 from its mesh's 32768-row image table with bulk SWDGE
``dma_gather`` ops (1024 rows per instruction -- the SWDGE descriptor ring
holds 1024 descriptors, which bounds indices per gather), then streams tiles
back to DRAM with contiguous HWDGE writes. The gather's stream->partition
mapping (i -> partition i%128, segment i//128) is inverted host-side in the
index layout so the writeback is a plain contiguous DMA.

int16 indices cover exactly 0..32767, so there are no spare indices for zero
rows: the top ZN=1024 table rows are sacrificed as zero rows (zero-reads must
be spread across many HBM banks -- a single hot row serializes the SDMA
engines, measured 3x slower), and the ~hundreds of output rows that genuinely
reference a sacrificed image row are patched on the host.
"""

import numpy as np

B = 4
N_FULL = 65536
N_SMALL = 32768
K = 32768
F = 256
P = 128
N_CORES = 8
HALF = N_FULL // 2          # output rows per core
NI = 1024                   # rows per dma_gather (SWDGE ring capacity)
SEG = NI // P               # segments per partition per op (8)
OPS = HALF // NI            # 32 ops per core
NBUF = 8                    # data tile double-buffering depth
NQ = 2                      # SWDGE queues (descriptor rings) gathers rotate over
ZN = 1024                   # sacrificed zero rows (spreads zero-reads across
ZBASE = N_SMALL - ZN        # HBM banks; one hot row serializes the SDMAs)

_prog_cache = {}


def _build_program(reps: int = 1, n_queues: int = 1):
    import concourse.bacc as bacc
    import concourse.tile as tile
    import concourse.mybir as mybir

    nc = bacc.Bacc(
        "TRN2", target_bir_lowering=False, num_swdge_queues=n_queues
    )
    img = nc.dram_tensor("img", [N_SMALL, F], mybir.dt.float32, kind="ExternalInput")
    gidx = nc.dram_tensor(
        "gidx", [P, OPS * NI // 16], mybir.dt.int16, kind="ExternalInput"
    )
    out = nc.dram_tensor("out", [HALF, F], mybir.dt.float32, kind="ExternalOutput")

    with tile.TileContext(nc) as tc:
        with (
            tc.tile_pool(name="data", bufs=NBUF) as pool,
            tc.tile_pool(name="idx", bufs=1) as ipool,
        ):
            idx_all = ipool.tile([P, OPS * NI // 16], mybir.dt.int16)
            nc.gpsimd.dma_start(out=idx_all[:], in_=gidx[:])
            for t in [u % OPS for u in range(reps * OPS)]:
                data = pool.tile([P, SEG * F], mybir.dt.float32)
                nc.gpsimd.dma_gather(
                    out_ap=data[:].rearrange("p (s f) -> p s f", f=F),
                    in_ap=img[:],
                    idxs_ap=idx_all[:, t * (NI // 16) : (t + 1) * (NI // 16)],
                    num_idxs=NI,
                    num_idxs_reg=NI,
                    elem_size=F,
                    queue_num=(t + 1) % n_queues,
                )
                # idx layout already inverted the gather's stream mapping, so
                # SBUF (p, s) holds output row t*NI + p*SEG + s: plain copy.
                nc.sync.dma_start(
                    out=out[t * NI : (t + 1) * NI, :].rearrange(
                        "(p s) f -> p (s f)", p=P
                    ),
                    in_=data[:],
                )
    nc.compile()  # EVSEM legalization: walrus allows 1 sync wait per inst
    return nc


def _get_program():
    # Two SWDGE queues: gathers alternate descriptor rings (ring capacity is
    # 1024 descriptors, exactly one gather), overlapping desc-gen with drain.
    # Measured ~218us/core vs ~288us single-queue; 4 queues (the ucode max)
    # is indistinguishable from 2 (219 vs 227us median slope on real data),
    # so the remaining ~20% over the 179us roofline is HBM random-read cost.
    if "nc" not in _prog_cache:
        _prog_cache["nc"] = _build_program(n_queues=NQ)
    return _prog_cache["nc"]


def _provenance(order_b: np.ndarray) -> np.ndarray:
    """src[i] = index into the initial scattered array that ends up at slot i."""
    fs = order_b[0, ::-1].tolist()
    ts = order_b[1, ::-1].tolist()
    src = list(range(N_FULL))
    for f, t in zip(fs, ts):
        src[t] = src[f]
    return np.asarray(src, dtype=np.int64)


def _idx_layout(g: np.ndarray) -> np.ndarray:
    """[HALF] row indices -> [P, OPS*NI/16] wrapped int16 dma_gather layout.

    Stream position i of op t must carry output row t*NI + (i%P)*SEG + i//P
    (so the writeback is contiguous); dma_gather reads stream position i from
    partition i%16, column i//16 (replicated across the 8 Q7 core groups).
    """
    stream = g.reshape(OPS, P, SEG).transpose(0, 2, 1).reshape(OPS, NI)
    w = stream.reshape(OPS, NI // 16, 16).transpose(2, 0, 1).reshape(16, -1)
    return np.ascontiguousarray(np.tile(w, (P // 16, 1)).astype(np.int16))


def _prep_mesh(images_b, mask_idx_b, order_b):
    """Returns (table, g_dev [N_FULL] int, patch_rows, patch_vals)."""
    src = _provenance(order_b)
    inv = np.full(N_FULL, -1, dtype=np.int32)
    inv[mask_idx_b] = np.arange(N_SMALL, dtype=np.int32)
    g0 = inv[src]  # [N_FULL]; -1 where the output row is zero
    # outputs that truly reference a sacrificed image row: patched on host
    patch_rows = np.where(g0 >= ZBASE)[0]
    patch_vals = images_b[g0[patch_rows]]
    # zero outputs gather from zero row ZBASE + (i % ZN): within each 1024-row
    # gather op every zero row is used at most once -> no hot HBM bank
    idx = np.arange(N_FULL, dtype=np.int32)
    g_dev = np.where(g0 < 0, ZBASE + (idx % ZN), g0)
    table = np.ascontiguousarray(images_b).copy()
    table[ZBASE:] = 0.0
    return table, g_dev, patch_rows, patch_vals


def _build_in_maps(images, mask_idx, order):
    images = np.ascontiguousarray(np.asarray(images, dtype=np.float32))
    mask_idx = np.asarray(mask_idx, dtype=np.int64)
    order = np.asarray(order, dtype=np.int64)
    assert images.shape == (B, N_SMALL, F)

    in_maps, patches = [], []
    for b in range(B):
        table, g_dev, patch_rows, patch_vals = _prep_mesh(
            images[b], mask_idx[b], order[b]
        )
        patches.append((patch_rows, patch_vals))
        for h in range(2):
            in_maps.append(
                {
                    "img": table,
                    "gidx": _idx_layout(g_dev[h * HALF : (h + 1) * HALF]),
                }
            )
    return in_maps, patches


def kernel(images, mask_idx, order, n_full, **run_kwargs):
    from concourse.bass_utils import run_bass_kernel_spmd

    assert int(n_full) == N_FULL
    in_maps, patches = _build_in_maps(images, mask_idx, order)
    res = run_bass_kernel_spmd(
        _get_program(), in_maps, core_ids=list(range(N_CORES)), **run_kwargs
    )
    outs = [res.results[c]["out"] for c in range(N_CORES)]
    full = np.stack(
        [np.concatenate([outs[2 * b], outs[2 * b + 1]], axis=0) for b in range(B)]
    )
    for b, (patch_rows, patch_vals) in enumerate(patches):
        if len(patch_rows):
            full[b, patch_rows] = patch_vals
    if run_kwargs:
        kernel.last_results = res
    return full
